# revision 17
# baseline (speedup 1.0000x reference)
"""Tensor-parallel multi-head attention for Trainium2 (8 NeuronCores).

Problem: B=2, T=2048, E=1024, H=16 heads of dim 64.
  q/k/v = einsum('hei,bte->hbti'); s = q@k^T/sqrt(T); p = softmax(s)
  att = p@v; out = concat_heads(att) @ Wo^T
Sharding: tensor-parallel over heads — 2 heads per core; Wo sharded along
its input axis; partial outputs summed on host.

Numerics: attention logits have std ~181 (unscaled randn weights), so the
QK^T path needs ~fp32 precision: exact fp16 hi/lo split for Q and the
QK^T; K kept to fp16 (pre-scaled by 1/sqrt(T) so S comes out of the PE
already scaled). V path / PV / Wo run in fp16.

Engine budget (HW-calibrated: fp16 matmul 2 rows/cycle; DVE reduce and
Act exp ~1 elem/cycle/lane): the row-max (DVE) and exp (Act) over the
16.8M logits/core are the structural costs. Everything else is arranged
around them: denominators batched per (b,head) and folded into the PV
evacuation via a GPSIMD-broadcast row of reciprocals (softmax
normalization costs no extra DVE/Act pass), phase-3 evacuation split
DVE/Act, x/out DMA issue on the SP queue.
"""

import sys

sys.path.insert(0, "/opt/trn_rl_repo")

import numpy as np
import ml_dtypes

import concourse.bass as bass
import concourse.mybir as mybir
import concourse.tile as tile
from concourse import bacc

BF16 = ml_dtypes.bfloat16
NF16 = np.float16

B, T, E = 2, 2048, 1024
H, I = 16, 64
NCORES = 8
HPC = H // NCORES            # heads per core = 2
BT = B * T                   # 4096
HI = HPC * I                 # 128 = per-core slice of the h*i axis
EC = E // 128                # 8 e-chunks
TB = T // 128                # 16 t-blocks per batch row
SCALE = 1.0 / float(np.sqrt(np.float32(T)))

F32 = mybir.dt.float32
BF = mybir.dt.bfloat16
FP16 = mybir.dt.float16

USE_HILO = True


def build_program(use_hilo: bool = USE_HILO, repeat: int = 1,
                  phase_limit: int = 3) -> bass.Bass:
    nc = bacc.Bacc("TRN2", target_bir_lowering=False, debug=False,
                   num_devices=NCORES)

    xh_d = nc.dram_tensor("xh", [E, BT], FP16, kind="ExternalInput")
    xl_d = nc.dram_tensor("xl", [E, BT], FP16, kind="ExternalInput")
    wqh_d = nc.dram_tensor("wqh", [128, EC, HI], FP16, kind="ExternalInput")
    wql_d = nc.dram_tensor("wql", [128, EC, HI], FP16, kind="ExternalInput")
    wkh_d = nc.dram_tensor("wkh", [128, EC, HI], FP16, kind="ExternalInput")
    wkl_d = nc.dram_tensor("wkl", [128, EC, HI], FP16, kind="ExternalInput")
    wv_d = nc.dram_tensor("wv", [128, EC, HI], FP16, kind="ExternalInput")
    wo_d = nc.dram_tensor("wo_t", [HI, E], FP16, kind="ExternalInput")
    if repeat > 1:
        # distinct per-rep output slots keep every iteration live (the
        # compiler dead-store-eliminates overwritten reps otherwise)
        out_d = nc.dram_tensor("out", [repeat, BT, E], FP16,
                               kind="ExternalOutput")
    else:
        out_d = nc.dram_tensor("out", [BT, E], FP16, kind="ExternalOutput")

    X = mybir.AxisListType.X

    with tile.TileContext(nc) as tc:
        with (
            tc.tile_pool(name="psum", bufs=8, space="PSUM") as psp,
            tc.tile_pool(name="xstream", bufs=4) as xp,
            tc.tile_pool(name="weights", bufs=1) as wp,
            tc.tile_pool(name="persist", bufs=1) as pk,
            tc.tile_pool(name="big", bufs=1) as bigp,
            tc.tile_pool(name="ptile", bufs=3) as ptp,
            tc.tile_pool(name="stats", bufs=8) as stp,
            tc.tile_pool(name="rbc", bufs=2) as rbp,
            tc.tile_pool(name="den", bufs=2) as dnp,
            tc.tile_pool(name="evac", bufs=3) as evp,
        ):
            # --- load weights into SBUF ---
            wqh = wp.tile([128, EC, HI], FP16, tag="wqh")
            wql = wp.tile([128, EC, HI], FP16, tag="wql")
            wkh = wp.tile([128, EC, HI], FP16, tag="wkh")
            wkl = wp.tile([128, EC, HI], FP16, tag="wkl")
            nc.sync.dma_start(wqh[:], wqh_d[:])
            nc.sync.dma_start(wql[:], wql_d[:])
            nc.sync.dma_start(wkh[:], wkh_d[:])
            nc.sync.dma_start(wkl[:], wkl_d[:])
            wv = wp.tile([128, EC, HI], FP16, tag="wv")
            nc.sync.dma_start(wv[:], wv_d[:])
            wo = wp.tile([128, E], FP16, tag="wo")
            nc.sync.dma_start(wo[:], wo_d[:])

            for _rep in range(repeat):
                # --- persistent activations ---
                Qh = pk.tile([128, BT], FP16, tag="Qh")
                Ql = pk.tile([128, BT], FP16, tag="Ql")
                K16 = pk.tile([128, BT], FP16, tag="K16")  # pre-scaled
                V = pk.tile([128, BT // 128, 130], FP16, tag="V")
                nc.gpsimd.memset(V[:, :, 64:65], 1.0)
                nc.gpsimd.memset(V[:, :, 129:130], 1.0)
                OT = pk.tile([128, BT], FP16, tag="OT")    # normalized

                # ================= Phase 1: QKV projections =================
                # Q^T[i, t] = sum_e W[e, i] * xT[e, t]; t-banks of 512.
                for tb8 in range(BT // 512):
                    ts = slice(tb8 * 512, (tb8 + 1) * 512)
                    qt_ps = psp.tile([128, 512], F32, tag="ps")
                    kt_ps = psp.tile([128, 512], F32, tag="ps")
                    xbh = [None, None]
                    xbl = [None, None]
                    for h4 in range(2):
                        sl4 = slice(h4 * 512, (h4 + 1) * 512)
                        xbh[h4] = xp.tile([128, 4, 512], FP16, tag="xbh",
                                          name=f"xbh_{h4}")
                        nc.sync.dma_start(
                            xbh[h4][:],
                            xh_d[sl4, ts].rearrange("(o p) t -> p o t", p=128))
                        xbl[h4] = xp.tile([128, 4, 512], FP16, tag="xbl",
                                          name=f"xbl_{h4}")
                        nc.sync.dma_start(
                            xbl[h4][:],
                            xl_d[sl4, ts].rearrange("(o p) t -> p o t", p=128))
                    v_ps = [psp.tile([128, 128], F32, tag="ps",
                                     name=f"v_ps_{c4}") for c4 in range(4)]
                    for ec in range(EC):
                        xhs = xbh[ec // 4][:, ec % 4, :]
                        xls = xbl[ec // 4][:, ec % 4, :]
                        # Q^T += Wh'xh + Wh'xl + Wl'xh  (and same for K)
                        nc.tensor.matmul(qt_ps[:], wqh[:, ec, :], xhs[:],
                                         start=(ec == 0), stop=False)
                        nc.tensor.matmul(qt_ps[:], wqh[:, ec, :], xls[:],
                                         start=False, stop=False)
                        nc.tensor.matmul(qt_ps[:], wql[:, ec, :], xhs[:],
                                         start=False, stop=(ec == EC - 1))
                        nc.tensor.matmul(kt_ps[:], wkh[:, ec, :], xhs[:],
                                         start=(ec == 0), stop=False)
                        nc.tensor.matmul(kt_ps[:], wkh[:, ec, :], xls[:],
                                         start=False, stop=False)
                        nc.tensor.matmul(kt_ps[:], wkl[:, ec, :], xhs[:],
                                         start=False, stop=(ec == EC - 1))
                        # V directly in [s, i] layout: x^T chunk stationary,
                        # wv moving — no transpose needed
                        for c4 in range(4):
                            nc.tensor.matmul(
                                v_ps[c4][:],
                                xbh[ec // 4][:, ec % 4,
                                             c4 * 128:(c4 + 1) * 128],
                                wv[:, ec, :],
                                start=(ec == 0), stop=(ec == EC - 1))

                    # evacuate; Q via hi/lo split, K cast with SCALE folded in
                    nc.scalar.copy(Qh[:, ts], qt_ps[:])
                    nc.vector.tensor_tensor(Ql[:, ts], qt_ps[:], Qh[:, ts],
                                            mybir.AluOpType.subtract)
                    nc.scalar.mul(K16[:, ts], kt_ps[:], SCALE)
                    for c4 in range(4):
                        nc.scalar.copy(V[:, tb8 * 4 + c4, 0:64],
                                       v_ps[c4][:, 0:64])
                        nc.scalar.copy(V[:, tb8 * 4 + c4, 65:129],
                                       v_ps[c4][:, 64:128])

                # ============ Phase 2: attention per (b, head) ============
                def emit_pv(b, hh, hr, PT, rcpb):
                    # PV: O^T[i, t-bank] = sum_s V'[s, i|1] * P^T[s, t];
                    # row 64 of o_ps = sum_s P = softmax denominator.
                    # Evacuate unnormalized immediately (frees PSUM for the
                    # next head's QK), then one broadcast + in-place
                    # reciprocal + in-place normalize per (b,h).
                    drow = dnp.tile([1, T], FP16, tag="drow")
                    for nb in range(4):
                        bank = slice(b * T + nb * 512, b * T + (nb + 1) * 512)
                        o_ps = psp.tile([65, 512], F32, tag="ps")
                        for sc in range(TB):
                            nc.tensor.matmul(
                                o_ps[:], V[:, b * TB + sc,
                                           hh * 65:(hh + 1) * 65],
                                PT[:, sc, nb * 512:(nb + 1) * 512],
                                start=(sc == 0), stop=(sc == TB - 1))
                        nc.scalar.copy(drow[:, nb * 512:(nb + 1) * 512],
                                       o_ps[64:65, :])
                        nc.vector.tensor_copy(OT[hr, bank], o_ps[0:64, :])
                    nc.gpsimd.partition_broadcast(rcpb[:], drow[0:1, :])
                    with nc.allow_low_precision(
                            reason="1/d to fp16: 2^-11 rel on softmax "
                                   "scale, within error budget"):
                        nc.vector.reciprocal(rcpb[:], rcpb[:])
                    bhalf = slice(b * T, (b + 1) * T)
                    nc.vector.tensor_tensor(OT[hr, bhalf], OT[hr, bhalf],
                                            rcpb[hr, :],
                                            mybir.AluOpType.mult)

                pending_pv = None
                for b in range(B if phase_limit >= 2 else 0):
                    for hh in range(HPC):
                        hr = slice(hh * 64, (hh + 1) * 64)
                        PT = bigp.tile([128, TB, T], FP16, tag="PT")
                        for tb in range(TB):
                            if tb == 4 and pending_pv is not None:
                                emit_pv(*pending_pv)
                                pending_pv = None
                            tcols = slice(b * T + tb * 128,
                                          b * T + (tb + 1) * 128)
                            s_ps = [psp.tile([128, 512], F32, tag="ps",
                                             name=f"s_ps_{j}")
                                    for j in range(4)]
                            for pi, lh in enumerate((Qh, Ql)):
                                for j in range(4):
                                    scols = slice(b * T + j * 512,
                                                  b * T + (j + 1) * 512)
                                    nc.tensor.matmul(
                                        s_ps[j][:], lh[hr, tcols],
                                        K16[hr, scols],
                                        start=(pi == 0), stop=(pi == 1))
                            # row max (DVE) -> negated bias (S pre-scaled)
                            m4 = stp.tile([128, 4], F32, tag="m4")
                            for j in range(4):
                                nc.vector.reduce_max(m4[:, j:j + 1],
                                                     s_ps[j][:], axis=X)
                            negb = stp.tile([128, 1], F32, tag="negb")
                            nc.vector.reduce_max(negb[:], m4[:], axis=X,
                                                 negate=True)
                            # unnormalized P = exp(s - max); denominators
                            # come free from the PV ones-column
                            Pt = ptp.tile([128, T], FP16, tag="Pt")
                            for j in range(4):
                                nc.scalar.activation(
                                    Pt[:, j * 512:(j + 1) * 512], s_ps[j][:],
                                    mybir.ActivationFunctionType.Exp,
                                    bias=negb[:], scale=1.0)
                            # P block [t=128, s=T] -> P^T[s-in, s-chunk, t]
                            nc.sync.dma_start_transpose(
                                PT[:, :, tb * 128:(tb + 1) * 128], Pt[:])

                        rcpb = rbp.tile([128, T], FP16, tag="rcpb")
                        pending_pv = (b, hh, hr, PT, rcpb)
                if pending_pv is not None:
                    emit_pv(*pending_pv)
                    pending_pv = None

                # ============ Phase 3: output projection ============
                # out[t, e] = sum_i OT[i, t] * wo[i, e]
                for obp in range(BT // 256 if phase_limit >= 3 else 0):
                    o_sb = evp.tile([128, 2, E], FP16, tag="osb")
                    for oo in range(2):
                        ob = obp * 2 + oo
                        trows = slice(ob * 128, (ob + 1) * 128)
                        for eb in range(E // 512):
                            w_ps = psp.tile([128, 512], F32, tag="ps")
                            nc.tensor.matmul(w_ps[:], OT[:, trows],
                                             wo[:, eb * 512:(eb + 1) * 512],
                                             start=True, stop=True)
                            esl = slice(eb * 512, (eb + 1) * 512)
                            nc.scalar.copy(o_sb[:, oo, esl], w_ps[:])
                    od = (out_d[_rep] if repeat > 1 else out_d)
                    nc.sync.dma_start(
                        od[obp * 256:(obp + 1) * 256, :].rearrange(
                            "(o p) e -> p o e", p=128),
                        o_sb[:])
    nc.compile()
    return nc


def _split_fp16(a32: np.ndarray):
    hi = a32.astype(NF16)
    lo = (a32 - hi.astype(np.float32)).astype(NF16)
    return hi, lo


def make_in_maps(x, Wq, Wk, Wv, Wo, use_hilo: bool = USE_HILO):
    """Build the 8 per-core input maps from the full inputs."""
    x = np.asarray(x, np.float32)
    Wq = np.asarray(Wq, np.float32)
    Wk = np.asarray(Wk, np.float32)
    Wv = np.asarray(Wv, np.float32)
    Wo = np.asarray(Wo, np.float32)

    xt = np.ascontiguousarray(x.reshape(BT, E).T)          # [E, BT]
    xth16, xtl16 = _split_fp16(xt)
    in_maps = []
    for c in range(NCORES):
        hsl = slice(c * HPC, (c + 1) * HPC)

        # [E, HPC*I] -> [128, EC, HI] (partition-major)
        def _pmaj(w):
            return np.ascontiguousarray(
                w.reshape(EC, 128, HI).transpose(1, 0, 2))
        wq_c = _pmaj(np.concatenate(list(Wq[hsl]), axis=1))
        wk_c = _pmaj(np.concatenate(list(Wk[hsl]), axis=1))
        wv_c = _pmaj(np.concatenate(list(Wv[hsl]), axis=1))
        wo_c = np.ascontiguousarray(Wo[:, c * HI:(c + 1) * HI].T)  # [HI, E]
        m = {
            "wv": wv_c.astype(NF16),
            "wo_t": wo_c.astype(NF16),
            "xh": xth16, "xl": xtl16,
        }
        m["wqh"], m["wql"] = _split_fp16(wq_c)
        m["wkh"], m["wkl"] = _split_fp16(wk_c)
        in_maps.append(m)
    return in_maps


_CACHED = {}


def _get_program(use_hilo: bool = USE_HILO) -> bass.Bass:
    if use_hilo not in _CACHED:
        _CACHED[use_hilo] = build_program(use_hilo)
    return _CACHED[use_hilo]


def kernel(**inputs) -> np.ndarray:
    from concourse.bass_utils import run_bass_kernel_spmd

    nc = _get_program()
    in_maps = make_in_maps(inputs["x"], inputs["Wq"], inputs["Wk"],
                           inputs["Wv"], inputs["Wo"])
    res = run_bass_kernel_spmd(nc, in_maps, core_ids=list(range(NCORES)))
    out = np.zeros((BT, E), np.float32)
    for c in range(NCORES):
        out += np.asarray(res.results[c]["out"], np.float32)
    return out.reshape(B, T, E)
